# revision 1
# baseline (speedup 1.0000x reference)
"""Trainium2 Bass kernel for masked multi-head attention with adjacency-derived
sparse masks (nn_MultiHeadAttention_4922032521398).

Reference (per batch of 32, L=512, DIM=256, 4 heads x 64):
    qkv = x @ w_qkv.T ; q,k,v per head
    score = q @ k.T / sqrt(64)
    a   = binarize(adj): 1 where adj==1 or adj>=9 else 0
    pe  = stack([a, aT, aT@a, a@aT]) + I   (per-head masks, !=0 -> keep)
    out = softmax(where(pe==0, -inf, score)) @ v ; y = out @ w_proj.T

Strategy (data-parallel over batch across 8 cores, 4 batches each):
  - Scores built transposed: S^T[k,q] so attention@V and the projection
    contract without any on-device transposes.  P^T = exp(S^T/8)*mask^T;
    mask transposes are free (m0^T=(aT|I), m1^T=(a|I), m2/m3 symmetric).
    Scores are small (|s|<~2) so exp needs no max-subtraction, and the 0/1
    mask multiply equals -inf masking exactly.
  - Heads 2/3: adjacency counts (aT@a / a@aT as fp8 DoubleRow matmuls on the
    exact 0/1 values, fp32 PSUM accumulate => exact counts) stay in PSUM and
    fuse into the softmax as P^T=(count>=0.5)*exp(S^T/8) in one
    scalar_tensor_tensor op -- no materialized mask, no Sign pass.
  - Heads 0/1: bins get the identity OR'd in place (after the count matmuls
    consumed the pure bins) and serve directly as masks.
  - Row sums via a ones-column appended to V (PV matmul row 64), applied via
    reciprocal + K=1 broadcast matmul + elementwise multiply.
  - Host passes xT / adj(bf16) / adjT(bf16) so no device transposes.
  - QKV/proj matmuls in float32r (full-rate fp32); attention P/V in bf16.
  - Elementwise stages operate on 2-bank PSUM tiles ([128,2,512]) to halve
    per-op fixed costs on ACT/DVE.
"""

import os
import sys

os.environ.setdefault("JAX_PLATFORMS", "axon,cpu")

for _p in ("/opt/trn_rl_repo",):
    if _p not in sys.path:
        sys.path.append(_p)

import numpy as np
import ml_dtypes

import concourse.bass as bass
import concourse.mybir as mybir
import concourse.tile as tile
from concourse import bacc
from concourse.bass_utils import run_bass_kernel_spmd
from concourse.masks import make_identity

B, L, DIM, NH = 32, 512, 256, 4
HD = DIM // NH  # 64
SCALE = float(np.sqrt(HD))
NCORES = 8
BPC = B // NCORES  # batches per core

F32 = mybir.dt.float32
F32R = mybir.dt.float32r
BF16 = mybir.dt.bfloat16
FP8 = mybir.dt.float8e4
AF = mybir.ActivationFunctionType
OP = mybir.AluOpType
DR = mybir.MatmulPerfMode.DoubleRow

# knobs
MM_FAST = True      # float32r full-rate fp32 matmuls for qkv/proj
PT_BF16 = True      # P^T / V / exp outputs in bf16
MASK_FP8_DR = True  # fp8 DoubleRow for the adjacency count matmuls

FMM = F32R if MM_FAST else F32


def build_nc():
    nc = bacc.Bacc("TRN2", target_bir_lowering=False)
    xT_d = nc.declare_dram_parameter("xT", [BPC, DIM, L], FMM, isOutput=False)
    adj_d = nc.declare_dram_parameter("adjb", [BPC, L, L], BF16, isOutput=False)
    adjT_d = nc.declare_dram_parameter("adjbT", [BPC, L, L], BF16, isOutput=False)
    wqkvT_d = nc.declare_dram_parameter("wqkvT", [DIM, 3 * DIM], FMM, isOutput=False)
    wprojT_d = nc.declare_dram_parameter("wprojT", [DIM, DIM], FMM, isOutput=False)
    y_d = nc.declare_dram_parameter("y", [BPC, L, DIM], F32, isOutput=True)

    pt_dt = BF16 if PT_BF16 else FMM

    with tile.TileContext(nc) as tc:
        with (
            tc.tile_pool(name="const", bufs=1) as cpool,
            tc.tile_pool(name="inp", bufs=2) as ipool,
            tc.tile_pool(name="work", bufs=2) as wpool,
            tc.tile_pool(name="head", bufs=2) as hpool,
            tc.tile_pool(name="small", bufs=4) as spool,
            tc.tile_pool(name="psum", bufs=3, space="PSUM") as pspool,   # 2-bank slots
            tc.tile_pool(name="psumc", bufs=2, space="PSUM") as pcpool,  # 1-bank slots
        ):
            # ---- constants (loaded once) ----
            wqkvT_sb = cpool.tile([128, 2, 3 * DIM], FMM)  # [p, dchunk, o]
            nc.sync.dma_start(
                out=wqkvT_sb[:, :, :],
                in_=wqkvT_d[:, :].rearrange("(c p) o -> p c o", p=128),
            )
            wprojT_sb = cpool.tile([64, NH, DIM], FMM)  # per head on 64 parts
            nc.sync.dma_start(
                out=wprojT_sb[:, :, :],
                in_=wprojT_d[:, :].rearrange("(h p) o -> p h o", p=64),
            )
            ident_sb = cpool.tile([128, 128], BF16)
            make_identity(nc, ident_sb[:, :])
            ones_src = cpool.tile([128, HD], F32)
            nc.vector.memset(ones_src[:, :], 1.0)
            # dependency-free warm-up activation at kernel start: hoists the
            # ~2.7us exp_and_others ACT_TABLE_LOAD into the initial DMA ramp
            # instead of blocking the first real exp mid-stream (the cost
            # model underprices this load; hardware pays it once either way)
            act_warm = cpool.tile([1, 8], F32)
            nc.scalar.activation(act_warm[:, :], ones_src[0:1, 0:8], AF.Exp)
            # PE HAM warm-up: ~3.4us of dependency-free matmuls during the
            # initial DMA ramp lift the PE clock gate to 8/8 (2.4 GHz) before
            # the first real matmuls; otherwise they run the first ~3.4us at
            # half clock. Off the critical path; sink read defeats DCE.
            warm_ps = pcpool.tile([128, 128], F32, tag="cnt")
            for _w in range(48):
                nc.tensor.matmul(
                    warm_ps[:, :], lhsT=ident_sb[:, :], rhs=ident_sb[:, :],
                    start=True, stop=True,
                )
            warm_sink = cpool.tile([1, 8], F32)
            nc.scalar.copy(warm_sink[:, :], warm_ps[0:1, 0:8])
            # ones row at partition 64 (same base partition as the PV rowsum
            # row, so the K=1 broadcast matmul sees equal base partitions)
            ones_t = cpool.tile([65, HD], FMM)
            nc.scalar.copy(ones_t[64:65, :], ones_src[64:65, :])

            for b in range(BPC):
                # ---------- load ----------
                adj_sb = ipool.tile([128, 4, L], BF16)
                nc.sync.dma_start(
                    out=adj_sb[:, :, :],
                    in_=adj_d[b].rearrange("(c p) j -> p c j", p=128),
                )
                adjT_sb = ipool.tile([128, 4, L], BF16)
                nc.sync.dma_start(
                    out=adjT_sb[:, :, :],
                    in_=adjT_d[b].rearrange("(c p) j -> p c j", p=128),
                )
                xT_sb = ipool.tile([128, 2, L], FMM)  # x^T: [p, dchunk, l]
                nc.sync.dma_start(
                    out=xT_sb[:, :, :],
                    in_=xT_d[b].rearrange("(c p) l -> p c l", p=128),
                )

                # ---------- QK^T = w_qk @ x^T : [512(o), 512(l)] ----------
                # chunks 0..1 = Q^T (heads 0,1 | 2,3 by 64 rows), 2..3 = K^T
                qkt_sb = wpool.tile([128, 4, L], FMM)
                for op in range(2):  # pairs of output chunks
                    ps = pspool.tile([128, 2, L], F32, tag="ps")
                    for i in range(2):
                        oc = op * 2 + i
                        for c in range(2):
                            nc.tensor.matmul(
                                ps[:, i, :],
                                lhsT=wqkvT_sb[:, c, oc * 128:(oc + 1) * 128],
                                rhs=xT_sb[:, c, :],
                                start=(c == 0),
                                stop=(c == 1),
                            )
                    nc.scalar.copy(qkt_sb[:, op * 2:op * 2 + 2, :], ps[:, :, :])

                # ---------- V (natural layout) + ones column ----------
                v_sb = wpool.tile([128, 4, NH, HD + 1], pt_dt)
                nc.scalar.copy(
                    v_sb[:, :, :, HD:HD + 1],
                    ones_src[:, 0:16].rearrange("p (a b c) -> p a b c", a=4, b=NH),
                )
                for lp in range(2):  # pairs of l-chunks
                    psv = pcpool.tile([128, 2, NH * HD], F32, tag="cnt")
                    # one accumulation group for the whole (single-bank) tile:
                    # a second start=True would clear the bank's has_written
                    # bits and corrupt the first half under interleaving
                    for i in range(2):
                        lc = lp * 2 + i
                        for c in range(2):
                            nc.tensor.matmul(
                                psv[:, i, :],
                                lhsT=xT_sb[:, c, lc * 128:(lc + 1) * 128],
                                rhs=wqkvT_sb[:, c, 2 * DIM:3 * DIM],
                                start=(i == 0 and c == 0),
                                stop=(i == 1 and c == 1),
                                skip_group_check=True,
                            )
                    nc.scalar.copy(
                        v_sb[:, lp * 2:lp * 2 + 2, :, 0:HD],
                        psv[:, :, :].rearrange("p i (h d) -> p i h d", h=NH),
                    )

                # ---------- binarize adjacency: a = (adj==1)|(adj>=9) ----------
                abin_sb = wpool.tile([128, 4, L], BF16)
                aTbin_sb = wpool.tile([128, 4, L], BF16)
                for src, dst in ((adj_sb, abin_sb), (adjT_sb, aTbin_sb)):
                    for cp in range(2):  # pairs of chunks
                        sl2 = slice(cp * 2, cp * 2 + 2)
                        tmp = spool.tile([128, 2, L], BF16, tag="bintmp")
                        nc.vector.tensor_scalar(
                            tmp[:, :, :], src[:, sl2, :], 9.0, None, OP.is_ge
                        )
                        nc.vector.scalar_tensor_tensor(
                            dst[:, sl2, :],
                            in0=src[:, sl2, :],
                            scalar=1.0,
                            in1=tmp[:, :, :],
                            op0=OP.is_equal,
                            op1=OP.max,
                        )
                if MASK_FP8_DR:
                    # fp8 copies of the exact 0/1 bins for DoubleRow matmuls;
                    # split across ACT and DVE to balance engine load
                    abin8_sb = wpool.tile([128, 4, L], FP8)
                    aTbin8_sb = wpool.tile([128, 4, L], FP8)
                    for srcb, dst8, on_act in (
                        (abin_sb, abin8_sb, True),
                        (aTbin_sb, aTbin8_sb, True),
                    ):
                        for cp in range(2):
                            sl2 = slice(cp * 2, cp * 2 + 2)
                            if on_act:
                                nc.scalar.copy(dst8[:, sl2, :], srcb[:, sl2, :])
                            else:
                                nc.vector.tensor_copy(dst8[:, sl2, :], srcb[:, sl2, :])
                    mm_bins = (abin8_sb, aTbin8_sb)
                else:
                    mm_bins = (abin_sb, aTbin_sb)

                # ---------- attention ----------
                outTn_sb = wpool.tile([64, NH, L], FMM)  # normalized out^T

                def softmax_pv(h, pt_pair_fn):
                    """pt_pair_fn(kp, pss2, pt_sb): fill pt_sb[:, 2kp:2kp+2, :]
                    from the 2-chunk score psum pss2 [128, 2, L]."""
                    hp = slice((h % 2) * 64, (h % 2) * 64 + 64)
                    qc = h // 2
                    kc_ = 2 + h // 2
                    pt_sb = hpool.tile([128, 4, L], pt_dt, tag="pt")
                    for kp in range(2):
                        pss2 = pspool.tile([128, 2, L], F32, tag="ps")
                        for i in range(2):
                            kc = kp * 2 + i
                            nc.tensor.matmul(
                                pss2[:, i, :],
                                lhsT=qkt_sb[hp, kc_, kc * 128:(kc + 1) * 128],
                                rhs=qkt_sb[hp, qc, :],
                                start=True,
                                stop=True,
                            )
                        pt_pair_fn(kp, pss2, pt_sb)
                    # [V|1]^T @ P^T: rows 0..63 = out^T, row 64 = rowsums
                    pv = pcpool.tile([HD + 1, L], F32, tag="cnt")
                    for kc in range(4):
                        nc.tensor.matmul(
                            pv[:, :],
                            lhsT=v_sb[:, kc, h, :],
                            rhs=pt_sb[:, kc, :],
                            start=(kc == 0),
                            stop=(kc == 3),
                        )
                    inv_t = spool.tile([65, L], FMM, tag="inv")
                    with nc.allow_low_precision(reason="f32r rowsum reciprocal"):
                        nc.vector.reciprocal(inv_t[64:65, :], pv[HD:HD + 1, :])
                    bc_ps = pcpool.tile([HD, L], F32, tag="cnt")
                    nc.tensor.matmul(
                        bc_ps[:, :],
                        lhsT=ones_t[64:65, :],
                        rhs=inv_t[64:65, :],
                        start=True,
                        stop=True,
                    )
                    bc_sb = spool.tile([HD, L], F32, tag="bc")
                    nc.scalar.copy(bc_sb[:, :], bc_ps[:, :])
                    nc.vector.tensor_mul(
                        outTn_sb[:, h, :], pv[0:HD, :], bc_sb[:, :]
                    )

                # heads 2/3 first (consume pure bins via fp8 count matmuls)
                for h, srcb in ((2, mm_bins[0]), (3, mm_bins[1])):
                    def pair23(kp, pss2, pt_sb, srcb=srcb):
                        cnt = pspool.tile([128, 2, L], F32, tag="ps")
                        for i in range(2):
                            kc = kp * 2 + i
                            if MASK_FP8_DR:
                                for kk in (0, 2):
                                    nc.tensor.matmul(
                                        cnt[:, i, :],
                                        lhsT=srcb[:, kk:kk + 2, kc * 128:(kc + 1) * 128],
                                        rhs=srcb[:, kk:kk + 2, :],
                                        start=(kk == 0),
                                        stop=False,
                                        perf_mode=DR,
                                    )
                            else:
                                for kk in range(4):
                                    nc.tensor.matmul(
                                        cnt[:, i, :],
                                        lhsT=srcb[:, kk, kc * 128:(kc + 1) * 128],
                                        rhs=srcb[:, kk, :],
                                        start=(kk == 0),
                                        stop=False,
                                    )
                            nc.tensor.matmul(
                                cnt[:, i, kc * 128:(kc + 1) * 128],
                                lhsT=ident_sb[:, :],
                                rhs=ident_sb[:, :],
                                start=False,
                                stop=True,
                                skip_group_check=True,
                            )
                        ex = spool.tile([128, 2, L], pt_dt, tag="ex")
                        nc.scalar.activation(
                            ex[:, :, :], pss2[:, :, :], AF.Exp, scale=1.0 / SCALE
                        )
                        # P^T = (count >= 0.5) * exp
                        nc.vector.scalar_tensor_tensor(
                            pt_sb[:, kp * 2:kp * 2 + 2, :],
                            in0=cnt[:, :, :],
                            scalar=0.5,
                            in1=ex[:, :, :],
                            op0=OP.is_ge,
                            op1=OP.mult,
                        )
                    softmax_pv(h, pair23)

                # heads 0/1: OR identity into bins in place, use as masks
                for srcb in (abin_sb, aTbin_sb):
                    for c in range(4):
                        sl = slice(c * 128, (c + 1) * 128)
                        nc.vector.tensor_tensor(
                            srcb[:, c, sl], srcb[:, c, sl], ident_sb[:, :], OP.max
                        )
                for h, mask in ((0, aTbin_sb), (1, abin_sb)):
                    def pair01(kp, pss2, pt_sb, mask=mask):
                        ex = spool.tile([128, 2, L], pt_dt, tag="ex")
                        nc.scalar.activation(
                            ex[:, :, :], pss2[:, :, :], AF.Exp, scale=1.0 / SCALE
                        )
                        nc.vector.tensor_mul(
                            pt_sb[:, kp * 2:kp * 2 + 2, :],
                            ex[:, :, :],
                            mask[:, kp * 2:kp * 2 + 2, :],
                        )
                    softmax_pv(h, pair01)

                # ---------- output projection ----------
                y_sb = wpool.tile([128, 4, DIM], F32)
                for lp in range(2):
                    psy = pcpool.tile([128, 2, DIM], F32, tag="cnt")
                    for i in range(2):
                        lc = lp * 2 + i
                        for h in range(NH):
                            nc.tensor.matmul(
                                psy[:, i, :],
                                lhsT=outTn_sb[:, h, lc * 128:(lc + 1) * 128],
                                rhs=wprojT_sb[:, h, :],
                                start=(i == 0 and h == 0),
                                stop=(i == 1 and h == NH - 1),
                                skip_group_check=True,
                            )
                    nc.scalar.copy(y_sb[:, lp * 2:lp * 2 + 2, :], psy[:, :, :])
                nc.sync.dma_start(
                    out=y_d[b].rearrange("(c p) o -> p c o", p=128),
                    in_=y_sb[:, :, :],
                )
    nc.compile()
    return nc


_CACHED = {}


def _get_nc():
    if "nc" not in _CACHED:
        _CACHED["nc"] = build_nc()
    return _CACHED["nc"]


def kernel(x, adj, w_qkv, w_proj, _want_results_obj=False, **run_kwargs):
    x = np.ascontiguousarray(np.asarray(x, dtype=np.float32))
    adj = np.asarray(adj)
    w_qkv = np.asarray(w_qkv, dtype=np.float32)
    w_proj = np.asarray(w_proj, dtype=np.float32)

    xT = np.ascontiguousarray(x.transpose(0, 2, 1))          # [B, DIM, L]
    adjb = adj.astype(ml_dtypes.bfloat16)                    # exact (0..15)
    adjbT = np.ascontiguousarray(adj.transpose(0, 2, 1)).astype(ml_dtypes.bfloat16)
    wqkvT = np.ascontiguousarray(w_qkv.T)                    # [DIM, 3*DIM]
    wprojT = np.ascontiguousarray(w_proj.T)                  # [DIM, DIM]

    in_maps = []
    for c in range(NCORES):
        sl = slice(c * BPC, (c + 1) * BPC)
        in_maps.append(
            {
                "xT": xT[sl],
                "adjb": adjb[sl],
                "adjbT": adjbT[sl],
                "wqkvT": wqkvT,
                "wprojT": wprojT,
            }
        )

    nc = _get_nc()
    res = run_bass_kernel_spmd(nc, in_maps, list(range(NCORES)), **run_kwargs)
    y = np.concatenate([res.results[c]["y"] for c in range(NCORES)], axis=0)
    if _want_results_obj:
        return y, res
    return y

